# revision 33
# baseline (speedup 1.0000x reference)
"""BiMamba (bidirectional Mamba-1 selective scan) on 8 Trainium2 NeuronCores.

Sharding: core c = (b, dir, half) with b = c>>2, dir = (c>>1)&1, half = c&1.
Each core computes its half of d_inner for one (batch, direction) in a
transposed [d, L] layout, fp16 matmul inputs / fp32 accumulation.

L is processed in chunks [512, 1024, 512] with per-(d,n) state carry so the
DVE scan stream starts early and the out_proj tail is short.  Phases are
generators interleaved by an explicit schedule: while the DVE works through
chunk c's scans, the PE runs chunk c+1's in_proj/conv/x_proj and chunk
c-1's out_proj.  All elementwise multiplies (dBu, h*C, dtu, gate) run on
the DVE: the GPSIMD shares SBUF ports with it, and any concurrent Pool
tensor op slows the DVE ~4x (scans ~1.9x), so the Pool is left idle.
  per chunk: in_proj -> depthwise conv (diagonal-weight matmuls) -> silu ->
  x_proj partial -> pairwise AllReduce of x_dbl (f16) -> dt softplus ->
  scan: dA = exp(A*dt) fp16 on ACT, dBu = dtu*B on DVE,
        h = tensor_tensor_scan on DVE, tmp = h*C on DVE,
        y accumulated on the PE with identity matmuls into PSUM
  -> gate with silu(z) in place -> out_proj partial -> output slice.
Host sums the pairwise partial outputs, transposes, and flips bwd.
"""
import sys
sys.path.insert(0, "/opt/trn_rl_repo")
import numpy as np
from contextlib import ExitStack

import concourse.bass as bass
import concourse.mybir as mybir
import concourse.tile as tile
from concourse.vector_clock import ScopedClock

F32 = mybir.dt.float32
F16 = mybir.dt.float16
AF = mybir.ActivationFunctionType
OP = mybir.AluOpType

# ---------------------------------------------------------------- geometry
B, L, DM = 2, 2048, 1024
DI, DS, DC, DTR = 2 * DM, 16, 4, DM // 16
DH = DI // 2              # d_inner half per core
NT = DH // 128            # d-tiles per core
MMT = 512                 # matmul free-dim tile
P = 128
KT = DM // P              # d_model tiles

CH = [512, 1024, 512]     # L chunks (small head -> scans start early)
NCH = len(CH)
CHO = [sum(CH[:i]) for i in range(NCH)]   # chunk offsets
LCMAX = max(CH)

# smalls packing (columns of the [128, SMALLW] f32 tensor)
SM_CONVB = 0              # NT cols
SM_DTB = SM_CONVB + NT    # NT cols
SM_DCOL = SM_DTB + NT     # NT cols
SM_A = SM_DCOL + NT       # NT*DS cols
SM_CONVW = SM_A + NT * DS # NT*DC cols
SMALLW = SM_CONVW + NT * DC

# wcat packing (columns of the [DM, WCATW] f16 tensor): w_in | w_out | w_x
WC_WIN = 0                # 2*DH cols
WC_WOUT = WC_WIN + 2 * DH # DM cols
WC_WX = WC_WOUT + DM      # 96 cols
WCATW = WC_WX + 96

MAXW = 1                  # codegen limit: sem waits per instruction


# ------------------------------------------------------------- tile patch
def _patched_drain_and_barrier(self, tick_clock, wait_clock):
    nop_inst = self.nc.sync.nop(nofuse=True)
    wait_clock.add_sem_waits(
        nop_inst.ins, ScopedClock({None: tick_clock.global_clock}))
    si = nop_inst.ins.sync_info
    if si is not None and si.on_wait and len(si.on_wait) > MAXW:
        extra = list(si.on_wait[MAXW:])
        del si.on_wait[MAXW:]
        for i in range(0, len(extra), MAXW):
            nop2 = self.nc.sync.nop(nofuse=True)
            nop2.ins.sync_info = mybir.SyncInfo(
                on_wait=extra[i:i + MAXW], on_update=[])
    self.nc.sync.drain()
    self.nc.all_engine_barrier()
    assert self.sems is not None
    popped = self.nc._tile_sem_poison_stack.pop()
    assert popped is self._sem_poison
    self.nc.clear_and_free_semaphores(list(self.sems.allocated().values()))
    self.nc.all_engine_barrier()


tile.TileContext._drain_and_barrier = _patched_drain_and_barrier


def split_multiwaits(nc, maxw=MAXW):
    ctr = 0
    for fn in nc.m.functions:
        for blk in fn.blocks:
            il = list(blk.instructions)
            out = []
            changed = False
            for ins in il:
                si = getattr(ins, "sync_info", None)
                waits = list(si.on_wait) if (si is not None and si.on_wait) else []
                if len(waits) > maxw:
                    changed = True
                    extra, keep = waits[:-maxw], waits[-maxw:]
                    for i in range(0, len(extra), maxw):
                        nop = mybir.InstNoOp(name=f"wsplit_{ctr}", ins=[], outs=[])
                        ctr += 1
                        nop.engine = ins.engine
                        nop.sync_info = mybir.SyncInfo(
                            on_wait=extra[i:i + maxw], on_update=[])
                        out.append(nop)
                    si.on_wait = keep
                out.append(ins)
            if changed:
                blk.instructions = out
    return ctr


def drive(gen, n):
    for _ in range(n):
        if next(gen, StopIteration) is StopIteration:
            return False
    return True


def drive_all(gen):
    for _ in gen:
        pass


def interleave(fg, bg_gens, per_yield):
    """Advance up to per_yield background units at each foreground yield,
    draining any leftovers afterwards.

    The foreground (s6) yields only at PSUM-group boundaries, so background
    matmuls are never issued inside an open accumulation group and every
    background op's dependencies resolve strictly backward in each engine
    queue (no cross-engine forward waits -> no deadlock)."""
    import itertools
    bg = itertools.chain(*bg_gens)
    for _ in fg:
        for _ in range(per_yield):
            next(bg, None)
    for _ in bg:
        pass


# ------------------------------------------------------------ bass builder
def build_nc():
    nc = bass.Bass()

    XH_N = DM * L
    WCH_N = DM * WCATW
    WDT_N = DTR * DH
    pk_d = nc.declare_dram_parameter("pk", [XH_N + WCH_N + WDT_N], F16,
                                     isOutput=False)
    xh_d = pk_d[0:XH_N].rearrange("(r c) -> r c", c=L)
    wch_d = pk_d[XH_N:XH_N + WCH_N].rearrange("(r c) -> r c", c=WCATW)
    wdt_d = pk_d[XH_N + WCH_N:XH_N + WCH_N + WDT_N].rearrange(
        "(k c) -> k c", c=DH)
    sm_d = nc.declare_dram_parameter("smalls", [P, SMALLW], F32, isOutput=False)
    outp_d = nc.declare_dram_parameter("outp", [DM, L], F16, isOutput=True)

    ccin = [nc.dram_tensor(f"ccin{c}", [96, CH[c]], F16) for c in range(NCH)]
    ccout = [nc.dram_tensor(f"ccout{c}", [96, CH[c]], F16) for c in range(NCH)]
    pairs = [[0, 1], [2, 3], [4, 5], [6, 7]]

    with tile.TileContext(nc) as tc, ExitStack() as ctx:
        pool = ctx.enter_context(tc.tile_pool(name="sb", bufs=1))
        psum = ctx.enter_context(tc.tile_pool(name="ps", bufs=6, space="PSUM"))

        # resident small weights
        wx_r = pool.tile([P, NT, 96], F16, tag="wx")
        nc.sync.dma_start(
            wx_r[:],
            wch_d[:, WC_WX:WC_WX + 96].rearrange("(kt p) m -> p kt m", p=P))
        wdt_r = pool.tile([DTR, NT, P], F16, tag="wdt")
        nc.sync.dma_start(wdt_r[:], wdt_d.rearrange("k (mt m) -> k mt m", m=P))
        sm = pool.tile([P, SMALLW], F32, tag="sm")
        nc.sync.dma_start(sm[:], sm_d[:])

        # depthwise-conv diagonal weights, built on device
        dmask = pool.tile([P, P], F16, tag="dmask")
        nc.gpsimd.memset(dmask[:], 1.0)
        nc.gpsimd.affine_select(
            out=dmask[:], in_=dmask[:], compare_op=OP.is_equal, fill=0.0,
            base=0, pattern=[[-1, P]], channel_multiplier=1)
        cdiag = []
        for nt in range(NT):
            cd = pool.tile([P, DC, P], F16, tag=f"cd{nt}", name=f"cd{nt}")
            for k in range(DC):
                nc.vector.tensor_scalar_mul(
                    cd[:, k, :], dmask[:],
                    sm[:, SM_CONVW + nt * DC + k:SM_CONVW + nt * DC + k + 1])
            cdiag.append(cd)

        halo = [pool.tile([P, DC - 1], F16, tag=f"halo{nt}", name=f"halo{nt}")
                for nt in range(NT)]
        states = pool.tile([P, DS * NT], F32, tag="states")

        xt_re = xh_d.rearrange("(kt p) l -> p kt l", p=P)
        st = [dict() for _ in range(NCH)]

        def s1(c):
            """in_proj: xi tiles (mt < NT) first, then z/sz tiles."""
            lc, l0, ltn = CH[c], CHO[c], CH[c] // MMT
            xt_t = []
            for kt in range(KT):
                t = pool.tile([P, lc], F16, tag="big", bufs=8,
                              name=f"xt_{c}_{kt}")
                nc.sync.dma_start(t[:], xt_re[:, kt, l0:l0 + lc])
                xt_t.append(t)
            st[c]["xt_t"] = xt_t
            xi_t, sz_t = [], []
            for mt in range(2 * NT):
                win_t = pool.tile([P, KT, P], F16, tag="win", bufs=4,
                                  name=f"win_{c}_{mt}")
                nc.sync.dma_start(
                    win_t[:],
                    wch_d[:, WC_WIN + mt * P:WC_WIN + (mt + 1) * P].rearrange(
                        "(kt p) q -> p kt q", p=P))
                if mt < NT:
                    xi = pool.tile([P, DC - 1 + lc], F16, tag="xi", bufs=8,
                                   name=f"xi_{c}_{mt}")
                    xi_t.append(xi)
                else:
                    sz = pool.tile([P, lc], F16, tag=f"sz{c}", bufs=NT,
                                   name=f"sz_{c}_{mt}")
                    sz_t.append(sz)
                for lt in range(ltn):
                    acc = psum.tile([P, MMT], F32, tag="mm", bufs=3,
                                    name=f"acc1_{c}_{mt}_{lt}")
                    for kt in range(KT):
                        nc.tensor.matmul(
                            acc[:], win_t[:, kt, :],
                            xt_t[kt][:, lt * MMT:(lt + 1) * MMT],
                            start=(kt == 0), stop=(kt == KT - 1))
                    if mt < NT:
                        nc.scalar.copy(
                            xi_t[mt][:, DC - 1 + lt * MMT:DC - 1 + (lt + 1) * MMT],
                            acc[:])
                    else:
                        nc.scalar.activation(
                            sz_t[mt - NT][:, lt * MMT:(lt + 1) * MMT],
                            acc[:], AF.Silu)
                if mt == NT - 1:
                    st[c].update(xi_t=xi_t)
                yield
            st[c].update(sz_t=sz_t)

        def s23(c):
            """Depthwise conv + bias + silu -> u; x_proj partial -> AllReduce."""
            lc, ltn = CH[c], CH[c] // MMT
            xi_t = st[c]["xi_t"]
            u_t = []
            for nt in range(NT):
                if c == 0:
                    nc.gpsimd.memset(halo[nt][:], 0.0)
                # ACT copies: keep the DVE queue free of background ops
                nc.scalar.copy(xi_t[nt][:, 0:DC - 1], halo[nt][:])
                u = pool.tile([P, lc], F16, tag="xi", bufs=8,
                              name=f"u_{c}_{nt}")
                for lt in range(ltn):
                    acc = psum.tile([P, MMT], F32, tag="mm", bufs=3,
                                    name=f"acc2_{c}_{nt}_{lt}")
                    for k in range(DC):
                        nc.tensor.matmul(
                            acc[:], cdiag[nt][:, k, :],
                            xi_t[nt][:, lt * MMT + k:lt * MMT + k + MMT],
                            start=(k == 0), stop=(k == DC - 1))
                    nc.scalar.activation(
                        u[:, lt * MMT:(lt + 1) * MMT], acc[:], AF.Silu,
                        bias=sm[:, SM_CONVB + nt:SM_CONVB + nt + 1])
                # save halo for the next chunk (before xi slot recycles)
                nc.scalar.copy(
                    halo[nt][:], xi_t[nt][:, lc:lc + DC - 1])
                u_t.append(u)
                yield
            # x_proj partial [96, lc] -> pairwise AllReduce (async)
            xdblp = pool.tile([96, lc], F16, tag="xdblp", bufs=2,
                              name=f"xdblp_{c}")
            for lt in range(ltn):
                acc96 = psum.tile([96, MMT], F32, tag="mm96", bufs=1,
                                  name=f"acc96_{c}_{lt}")
                for nt in range(NT):
                    nc.tensor.matmul(
                        acc96[:], wx_r[:, nt, :],
                        u_t[nt][:, lt * MMT:(lt + 1) * MMT],
                        start=(nt == 0), stop=(nt == NT - 1))
                nc.scalar.copy(xdblp[:, lt * MMT:(lt + 1) * MMT], acc96[:])
                yield
            dma_in = nc.sync.dma_start(ccin[c][:], xdblp[:])
            cc = nc.gpsimd.collective_compute(
                "AllReduce", OP.add, replica_groups=pairs,
                ins=[ccin[c][:]], outs=[ccout[c][:]])
            tile.add_dep_helper(cc.ins, dma_in.ins, reason="cc after dma_in")
            st[c].update(u_t=u_t, cc=cc)
            yield

        def s5(c):
            """dt = softplus(Wdt@dtr + b); dtu = dt*u; y = D*u."""
            lc, ltn = CH[c], CH[c] // MMT
            u_t = st[c]["u_t"]
            xdbl = pool.tile([96, lc], F16, tag="xdbl", bufs=2,
                             name=f"xdbl_{c}")
            dma_out = nc.sync.dma_start(xdbl[:], ccout[c][:])
            tile.add_dep_helper(dma_out.ins, st[c]["cc"].ins,
                                reason="read after cc")
            st[c]["xdbl"] = xdbl
            yield
            dt_t, dtu_t, y_t = [], [], []
            for nt in range(NT):
                dt = pool.tile([P, lc], F16, tag="dt", bufs=8,
                               name=f"dt_{c}_{nt}")
                for lt in range(ltn):
                    acc = psum.tile([P, MMT], F32, tag="mm", bufs=3,
                                    name=f"acc5_{c}_{nt}_{lt}")
                    nc.tensor.matmul(
                        acc[:], wdt_r[:, nt, :],
                        xdbl[0:DTR, lt * MMT:(lt + 1) * MMT],
                        start=True, stop=True)
                    e = pool.tile([P, MMT], F32, tag="spe", bufs=1,
                                  name=f"spe_{c}_{nt}_{lt}")
                    nc.scalar.activation(e[:], acc[:], AF.Exp,
                                         bias=sm[:, SM_DTB + nt:SM_DTB + nt + 1])
                    nc.scalar.activation(
                        dt[:, lt * MMT:(lt + 1) * MMT], e[:], AF.Ln, bias=1.0)
                dt_t.append(dt)
                y = pool.tile([P, lc], F16, tag="y", bufs=8,
                              name=f"y_{c}_{nt}")
                nc.scalar.mul(y[:], u_t[nt][:],
                              sm[:, SM_DCOL + nt:SM_DCOL + nt + 1])  # y = D*u
                y_t.append(y)
                yield
            st[c].update(dt_t=dt_t, y_t=y_t)

        def s6(c):
            """Selective scan; y accumulated on the PE via identity matmuls.
            dA on ACT (fp16), everything elementwise on the DVE."""
            lc, ltn = CH[c], CH[c] // MMT
            dt_t, y_t = st[c]["dt_t"], st[c]["y_t"]
            u_t = st[c]["u_t"]
            # dtu prologue on the DVE (foreground, after s5 fully drained)
            dtu_t = []
            for nt in range(NT):
                dtu = pool.tile([P, lc], F16, tag="dtu", bufs=8,
                                name=f"dtu_{c}_{nt}")
                nc.vector.tensor_tensor(dtu[:], dt_t[nt][:], u_t[nt][:],
                                        OP.mult)
                dtu_t.append(dtu)
            yield
            ynew_t = [None] * NT
            LAG = 2
            PF = 2
            for g in range(NT // 2):
                nts = (2 * g, 2 * g + 1)
                yp = [[psum.tile([P, MMT], F32, tag=f"yp{j}{lt}", bufs=1,
                                 name=f"yp_{c}_{g}_{j}_{lt}")
                       for lt in range(ltn)] for j in range(2)]
                for j, nt in enumerate(nts):
                    for lt in range(ltn):
                        nc.tensor.matmul(
                            yp[j][lt][:], dmask[:],
                            y_t[nt][:, lt * MMT:(lt + 1) * MMT],
                            start=True, stop=False, skip_group_check=True)
                bb_q, cb_q, h_q = {}, {}, {}

                def bcast(n):
                    # bb per state; C broadcasts land in per-PAIR tiles so
                    # the h*C multiply below runs as one [P, 2*lc] op.
                    bb = pool.tile([P, lc], F16, tag="bb", bufs=PF + 1,
                                   name=f"bb_{c}_{g}_{n}")
                    nc.sync.dma_start(
                        bb[:], ccout[c][DTR + n:DTR + n + 1, :]
                        .partition_broadcast(P))
                    bb_q[n] = bb
                    p, k = n // 2, n % 2
                    if k == 0:
                        cb_q[p] = pool.tile([P, 2, lc], F16, tag="cb", bufs=3,
                                            name=f"cb_{c}_{g}_{p}")
                    nc.sync.dma_start(
                        cb_q[p][:, k, :],
                        ccout[c][DTR + DS + n:DTR + DS + n + 1, :]
                        .partition_broadcast(P))

                for n in range(min(PF, DS)):
                    bcast(n)
                for n in range(DS + LAG):
                    if n < DS:
                        if n + PF < DS:
                            bcast(n + PF)
                        p, k = n // 2, n % 2
                        if k == 0:
                            h_q[p] = [
                                pool.tile([P, 2, lc], F16, tag="h16", bufs=3,
                                          name=f"hp_{c}_{g}_{p}_{j}")
                                for j in range(2)]
                        h_t = []
                        for j, nt in enumerate(nts):
                            dA = pool.tile([P, lc], F16, tag="dA", bufs=3,
                                           name=f"dA_{c}_{g}_{n}_{j}")
                            nc.scalar.activation(
                                dA[:], dt_t[nt][:], AF.Exp,
                                scale=sm[:, SM_A + nt * DS + n:
                                         SM_A + nt * DS + n + 1])
                            dBu = pool.tile([P, lc], F16, tag="dbu", bufs=3,
                                            name=f"dbu_{c}_{g}_{n}_{j}")
                            nc.vector.tensor_tensor(dBu[:], dtu_t[nt][:],
                                                    bb_q[n][:], OP.mult)
                            h_t.append((dA, dBu))
                        for j, nt in enumerate(nts):
                            dA, dBu = h_t[j]
                            init = 0.0 if c == 0 else states[:, n * NT + nt:
                                                             n * NT + nt + 1]
                            nc.vector.tensor_tensor_scan(
                                h_q[p][j][:, k, :], dA[:], dBu[:], init,
                                OP.mult, OP.add)
                            if c < NCH - 1:
                                nc.scalar.copy(
                                    states[:, n * NT + nt:n * NT + nt + 1],
                                    h_q[p][j][:, k, lc - 1:lc])
                        bb_q.pop(n - PF, None)
                    m = n - LAG
                    if m >= 0 and m % 2 == 1:
                        p = m // 2
                        for j, nt in enumerate(nts):
                            tmp = pool.tile([P, 2, lc], F16, tag="tmp16",
                                            bufs=3,
                                            name=f"tmp_{c}_{g}_{p}_{j}")
                            nc.vector.tensor_tensor(tmp[:], h_q[p][j][:],
                                                    cb_q[p][:], OP.mult)
                            for k in range(2):
                                for lt in range(ltn):
                                    nc.tensor.matmul(
                                        yp[j][lt][:], dmask[:],
                                        tmp[:, k, lt * MMT:(lt + 1) * MMT],
                                        start=False, stop=(m == DS - 1
                                                           and k == 1),
                                        skip_group_check=True)
                        h_q.pop(p)
                        cb_q.pop(p)
                for j, nt in enumerate(nts):
                    ynew = pool.tile([P, lc], F16, tag="y", bufs=8,
                                     name=f"ynew_{c}_{g}_{j}")
                    for lt in range(ltn):
                        nc.scalar.copy(ynew[:, lt * MMT:(lt + 1) * MMT],
                                       yp[j][lt][:])
                    ynew_t[nts[j]] = ynew
                # yield only at group boundaries: all PSUM accumulation
                # groups are closed here, so interleaved background matmuls
                # never land inside an open group.
                yield
            st[c]["y_t"] = ynew_t

        def s78(c):
            """Gate (in place into sz) + out_proj partial -> output slice."""
            lc, l0, ltn = CH[c], CHO[c], CH[c] // MMT
            y_t, sz_t = st[c]["y_t"], st[c]["sz_t"]
            for nt in range(NT):
                nc.vector.tensor_tensor(sz_t[nt][:], y_t[nt][:], sz_t[nt][:],
                                        OP.mult)
            yield
            for mt in range(KT):
                wout_t = pool.tile([P, NT, P], F16, tag="wout", bufs=4,
                                   name=f"wout_{c}_{mt}")
                nc.sync.dma_start(
                    wout_t[:],
                    wch_d[:, WC_WOUT + mt * P:WC_WOUT + (mt + 1) * P].rearrange(
                        "(kt p) q -> p kt q", p=P))
                for lt in range(ltn):
                    acc = psum.tile([P, MMT], F32, tag="mm", bufs=3,
                                    name=f"acc8_{c}_{mt}_{lt}")
                    for kt in range(NT):
                        nc.tensor.matmul(
                            acc[:], wout_t[:, kt, :],
                            sz_t[kt][:, lt * MMT:(lt + 1) * MMT],
                            start=(kt == 0), stop=(kt == NT - 1))
                    o = pool.tile([P, MMT], F16, tag="op", bufs=2,
                                  name=f"o_{c}_{mt}_{lt}")
                    nc.scalar.copy(o[:], acc[:])
                    nc.sync.dma_start(
                        outp_d[mt * P:(mt + 1) * P,
                               l0 + lt * MMT:l0 + (lt + 1) * MMT], o[:])
                yield

        # ---- schedule: phase-level sequencing (deadlock-free: every op's
        # deps point backward in its engine queue).  Overlap comes from the
        # engine queues draining asynchronously: chunk c+1's in_proj/conv
        # matmuls are queued before chunk c's scan-window yp matmuls, so the
        # PE runs them while the DVE works through chunk c's scans.
        g1 = [s1(c) for c in range(NCH)]
        drive(g1[0], NT)                 # xi in_proj of chunk 0
        drive_all(s23(0))                # conv + x_proj + AllReduce
        drive_all(s5(0))                 # dt/y-seeds of chunk 0
        interleave(s6(0),
                   [g1[0],               # z/sz of chunk 0
                    g1[1],               # in_proj of chunk 1
                    s23(1)], 9)
        drive_all(s5(1))
        interleave(s6(1),
                   [g1[2],               # in_proj of chunk 2
                    s23(2)], 7)
        drive_all(s5(2))
        drive_all(s78(0))
        interleave(s6(2),
                   [s78(1)], 3)
        drive_all(s78(2))

    split_multiwaits(nc)
    return nc


# ------------------------------------------------------------- host side
def _prep_core_inputs(inputs, b, dir_, half):
    pre = "f_" if dir_ == 0 else "b_"
    x = np.asarray(inputs["x"][b], dtype=np.float32)          # [L, DM]
    if dir_ == 1:
        x = x[::-1]
    sl = slice(half * DH, (half + 1) * DH)

    w_in_full = np.asarray(inputs[pre + "in_proj_w"], np.float32)  # [2DI, DM]
    w_in = np.concatenate([w_in_full[sl], w_in_full[DI + half * DH:
                                                    DI + (half + 1) * DH]], 0)
    conv_w = np.asarray(inputs[pre + "conv_w"], np.float32)[sl, 0]  # [DH, DC]
    conv_b = np.asarray(inputs[pre + "conv_b"], np.float32)[sl]
    w_x = np.asarray(inputs[pre + "x_proj_w"], np.float32)[:, sl]   # [96, DH]
    w_dt = np.asarray(inputs[pre + "dt_proj_w"], np.float32)[sl]    # [DH, DTR]
    dt_b = np.asarray(inputs[pre + "dt_proj_b"], np.float32)[sl]
    A = -np.exp(np.asarray(inputs[pre + "A_log"], np.float32))[sl]  # [DH, DS]
    Dp = np.asarray(inputs[pre + "D"], np.float32)[sl]
    w_out = np.asarray(inputs[pre + "out_proj_w"], np.float32)[:, sl]  # [DM,DH]

    smalls = np.zeros((P, SMALLW), np.float32)
    smalls[:, SM_CONVB:SM_CONVB + NT] = conv_b.reshape(NT, P).T
    smalls[:, SM_DTB:SM_DTB + NT] = dt_b.reshape(NT, P).T
    smalls[:, SM_DCOL:SM_DCOL + NT] = Dp.reshape(NT, P).T
    smalls[:, SM_A:SM_A + NT * DS] = (
        A.reshape(NT, P, DS).transpose(1, 0, 2).reshape(P, NT * DS))
    smalls[:, SM_CONVW:SM_CONVW + NT * DC] = (
        conv_w.reshape(NT, P, DC).transpose(1, 0, 2).reshape(P, NT * DC))

    wcat = np.concatenate([w_in.T, w_out.T, w_x.T], axis=1)   # [DM, WCATW]
    xt = x.T                                                  # [DM, L]
    pk = np.concatenate([
        np.ascontiguousarray(xt).astype(np.float16).ravel(),
        np.ascontiguousarray(wcat).astype(np.float16).ravel(),
        np.ascontiguousarray(w_dt.T).astype(np.float16).ravel(),
    ])
    return {"pk": pk, "smalls": smalls}


_CACHE = {}


def _get_nc():
    if "nc" not in _CACHE:
        _CACHE["nc"] = build_nc()
    return _CACHE["nc"]


def _make_runner():
    """Jitted 8-core PJRT runner."""
    import jax
    from jax.sharding import Mesh, PartitionSpec
    from jax.experimental.shard_map import shard_map
    from concourse import bass2jax
    from concourse.bass2jax import _bass_exec_p, install_neuronx_cc_hook

    install_neuronx_cc_hook()
    nc = _get_nc()
    pname = nc.partition_id_tensor.name if nc.partition_id_tensor else None
    in_names, out_names, out_avals = [], [], []
    for alloc in nc.m.functions[0].allocations:
        if not isinstance(alloc, mybir.MemoryLocationSet):
            continue
        name = alloc.memorylocations[0].name
        if alloc.kind == "ExternalInput":
            if name != pname:
                in_names.append(name)
        elif alloc.kind == "ExternalOutput":
            out_names.append(name)
            out_avals.append(jax.core.ShapedArray(
                tuple(alloc.tensor_shape), mybir.dt.np(alloc.dtype)))
    all_names = in_names
    if pname is not None:
        all_names = all_names + [pname]

    def _body(*args):
        operands = list(args)
        if pname is not None:
            operands.append(bass2jax.partition_id_tensor())
        outs = _bass_exec_p.bind(
            *operands, out_avals=tuple(out_avals), in_names=tuple(all_names),
            out_names=tuple(out_names), lowering_input_output_aliases=(),
            sim_require_finite=False, sim_require_nnan=False, nc=nc)
        return tuple(outs)

    devices = jax.devices()[:8]
    mesh = Mesh(np.asarray(devices), ("core",))
    nin = len(in_names)
    fn = jax.jit(shard_map(
        _body, mesh=mesh, in_specs=(PartitionSpec("core"),) * nin,
        out_specs=(PartitionSpec("core"),) * len(out_names), check_rep=False),
        keep_unused=True)
    return fn, in_names, out_names, out_avals


def _get_runner():
    if "runner" not in _CACHE:
        _CACHE["runner"] = _make_runner()
    return _CACHE["runner"]


def _concat_inputs(in_maps):
    import jax
    from jax.sharding import Mesh, NamedSharding, PartitionSpec
    fn, in_names, out_names, out_avals = _get_runner()
    concat = [np.concatenate([np.asarray(m[k]) for m in in_maps], axis=0)
              for k in in_names]
    mesh = Mesh(np.asarray(jax.devices()[:8]), ("core",))
    shard = NamedSharding(mesh, PartitionSpec("core"))
    return [jax.device_put(a, shard) for a in concat]


def _run(in_maps):
    import jax
    fn, in_names, out_names, out_avals = _get_runner()
    args = _concat_inputs(in_maps)
    outs = [np.asarray(o) for o in fn(*args)]
    return [
        {k: outs[i].reshape(8, *out_avals[i].shape)[c]
         for i, k in enumerate(out_names)}
        for c in range(8)
    ]


def run_timed(in_maps, iters=5):
    """Steady-state per-invocation time: issue a batch of executions
    back-to-back, block once, divide. Min over rounds."""
    import time as _t
    import jax
    fn, *_ = _get_runner()
    args = _concat_inputs(in_maps)
    args2 = _concat_inputs(in_maps)
    jax.block_until_ready(fn(*args))
    batch = max(iters, 1536)
    best = float("inf")
    for _ in range(4):
        try:
            t0 = _t.perf_counter()
            o = None
            for i in range(batch):
                o = fn(*(args if i % 2 == 0 else args2))
            jax.block_until_ready(o)
            best = min(best, (_t.perf_counter() - t0) / batch)
        except Exception:
            if best != float("inf"):
                break
            raise
    return best


def make_in_maps(inputs):
    return [
        _prep_core_inputs(inputs, c >> 2, (c >> 1) & 1, c & 1)
        for c in range(8)
    ]


def kernel(**inputs):
    in_maps = make_in_maps(inputs)
    res = _run(in_maps)
    # guard against a rare first-call collective-init flake: run twice and
    # retry while the two executions disagree materially.
    for _ in range(3):
        res2 = _run(in_maps)
        d = max(np.abs(res[c]["outp"].astype(np.float32)
                       - res2[c]["outp"].astype(np.float32)).max()
                for c in range(8))
        if d < 1e-3:
            break
        res = res2
    out = np.zeros((B, L, 2 * DM), np.float32)
    for b in range(B):
        for dir_ in range(2):
            c0 = (b << 2) | (dir_ << 1)
            part = (res[c0]["outp"].astype(np.float32)
                    + res[c0 + 1]["outp"].astype(np.float32))  # [DM, L]
            if dir_ == 1:
                part = part[:, ::-1]
            out[b, :, dir_ * DM:(dir_ + 1) * DM] = part.T
    return out


# revision 34
# speedup vs baseline: 1.1143x; 1.1143x over previous
"""BiMamba (bidirectional Mamba-1 selective scan) on 8 Trainium2 NeuronCores.

Sharding: core c = (b, dir, half) with b = c>>2, dir = (c>>1)&1, half = c&1.
Each core computes its half of d_inner for one (batch, direction) in a
transposed [d, L] layout, fp16 matmul inputs / fp32 accumulation.

L is processed in chunks [512, 1024, 512] with per-(d,n) state carry so the
DVE scan stream starts early and the out_proj tail is short.  Phases are
generators interleaved by an explicit schedule: while the DVE works through
chunk c's scans, the PE runs chunk c+1's in_proj/conv/x_proj and chunk
c-1's out_proj.  All elementwise multiplies (dBu, h*C, dtu, gate) run on
the DVE: the GPSIMD shares SBUF ports with it, and any concurrent Pool
tensor op slows the DVE ~4x (scans ~1.9x), so the Pool is left idle.
  per chunk: in_proj -> depthwise conv (diagonal-weight matmuls) -> silu ->
  x_proj partial -> pairwise AllReduce of x_dbl (f16) -> dt softplus ->
  scan: dA = exp(A*dt) fp16 on ACT, dBu = dtu*B on DVE,
        h = tensor_tensor_scan on DVE, tmp = h*C on DVE,
        y accumulated on the PE with identity matmuls into PSUM
  -> gate with silu(z) in place -> out_proj partial -> output slice.
Host sums the pairwise partial outputs, transposes, and flips bwd.
"""
import sys
sys.path.insert(0, "/opt/trn_rl_repo")
import numpy as np
from contextlib import ExitStack

import concourse.bass as bass
import concourse.mybir as mybir
import concourse.tile as tile
from concourse.vector_clock import ScopedClock

F32 = mybir.dt.float32
F16 = mybir.dt.float16
AF = mybir.ActivationFunctionType
OP = mybir.AluOpType

# ---------------------------------------------------------------- geometry
B, L, DM = 2, 2048, 1024
DI, DS, DC, DTR = 2 * DM, 16, 4, DM // 16
DH = DI // 2              # d_inner half per core
NT = DH // 128            # d-tiles per core
MMT = 512                 # matmul free-dim tile
P = 128
KT = DM // P              # d_model tiles

CH = [512, 1024, 512]     # L chunks (small head -> scans start early)
NCH = len(CH)
CHO = [sum(CH[:i]) for i in range(NCH)]   # chunk offsets
LCMAX = max(CH)

# smalls packing (columns of the [128, SMALLW] f32 tensor)
SM_CONVB = 0              # NT cols
SM_DTB = SM_CONVB + NT    # NT cols
SM_DCOL = SM_DTB + NT     # NT cols
SM_A = SM_DCOL + NT       # NT*DS cols
SM_CONVW = SM_A + NT * DS # NT*DC cols
SMALLW = SM_CONVW + NT * DC

# wcat packing (columns of the [DM, WCATW] f16 tensor): w_in | w_out | w_x
WC_WIN = 0                # 2*DH cols
WC_WOUT = WC_WIN + 2 * DH # DM cols
WC_WX = WC_WOUT + DM      # 96 cols
WCATW = WC_WX + 96

MAXW = 1                  # codegen limit: sem waits per instruction


# ------------------------------------------------------------- tile patch
def _patched_drain_and_barrier(self, tick_clock, wait_clock):
    nop_inst = self.nc.sync.nop(nofuse=True)
    wait_clock.add_sem_waits(
        nop_inst.ins, ScopedClock({None: tick_clock.global_clock}))
    si = nop_inst.ins.sync_info
    if si is not None and si.on_wait and len(si.on_wait) > MAXW:
        extra = list(si.on_wait[MAXW:])
        del si.on_wait[MAXW:]
        for i in range(0, len(extra), MAXW):
            nop2 = self.nc.sync.nop(nofuse=True)
            nop2.ins.sync_info = mybir.SyncInfo(
                on_wait=extra[i:i + MAXW], on_update=[])
    self.nc.sync.drain()
    self.nc.all_engine_barrier()
    assert self.sems is not None
    popped = self.nc._tile_sem_poison_stack.pop()
    assert popped is self._sem_poison
    self.nc.clear_and_free_semaphores(list(self.sems.allocated().values()))
    self.nc.all_engine_barrier()


tile.TileContext._drain_and_barrier = _patched_drain_and_barrier


def split_multiwaits(nc, maxw=MAXW):
    ctr = 0
    for fn in nc.m.functions:
        for blk in fn.blocks:
            il = list(blk.instructions)
            out = []
            changed = False
            for ins in il:
                si = getattr(ins, "sync_info", None)
                waits = list(si.on_wait) if (si is not None and si.on_wait) else []
                if len(waits) > maxw:
                    changed = True
                    extra, keep = waits[:-maxw], waits[-maxw:]
                    for i in range(0, len(extra), maxw):
                        nop = mybir.InstNoOp(name=f"wsplit_{ctr}", ins=[], outs=[])
                        ctr += 1
                        nop.engine = ins.engine
                        nop.sync_info = mybir.SyncInfo(
                            on_wait=extra[i:i + maxw], on_update=[])
                        out.append(nop)
                    si.on_wait = keep
                out.append(ins)
            if changed:
                blk.instructions = out
    return ctr


def drive(gen, n):
    for _ in range(n):
        if next(gen, StopIteration) is StopIteration:
            return False
    return True


def drive_all(gen):
    for _ in gen:
        pass


def interleave(fg, bg_gens, per_yield):
    """Advance up to per_yield background units at each foreground yield,
    draining any leftovers afterwards.

    The foreground (s6) yields only at PSUM-group boundaries, so background
    matmuls are never issued inside an open accumulation group and every
    background op's dependencies resolve strictly backward in each engine
    queue (no cross-engine forward waits -> no deadlock)."""
    import itertools
    bg = itertools.chain(*bg_gens)
    for _ in fg:
        for _ in range(per_yield):
            next(bg, None)
    for _ in bg:
        pass


# ------------------------------------------------------------ bass builder
def build_nc():
    nc = bass.Bass()

    XH_N = DM * L
    WCH_N = DM * WCATW
    WDT_N = DTR * DH
    pk_d = nc.declare_dram_parameter("pk", [XH_N + WCH_N + WDT_N], F16,
                                     isOutput=False)
    xh_d = pk_d[0:XH_N].rearrange("(r c) -> r c", c=L)
    wch_d = pk_d[XH_N:XH_N + WCH_N].rearrange("(r c) -> r c", c=WCATW)
    wdt_d = pk_d[XH_N + WCH_N:XH_N + WCH_N + WDT_N].rearrange(
        "(k c) -> k c", c=DH)
    sm_d = nc.declare_dram_parameter("smalls", [P, SMALLW], F32, isOutput=False)
    outp_d = nc.declare_dram_parameter("outp", [DM, L], F16, isOutput=True)

    ccin = [nc.dram_tensor(f"ccin{c}", [96, CH[c]], F16) for c in range(NCH)]
    ccout = [nc.dram_tensor(f"ccout{c}", [96, CH[c]], F16) for c in range(NCH)]
    pairs = [[0, 1], [2, 3], [4, 5], [6, 7]]

    with tile.TileContext(nc) as tc, ExitStack() as ctx:
        pool = ctx.enter_context(tc.tile_pool(name="sb", bufs=1))
        psum = ctx.enter_context(tc.tile_pool(name="ps", bufs=6, space="PSUM"))

        # resident small weights
        wx_r = pool.tile([P, NT, 96], F16, tag="wx")
        nc.sync.dma_start(
            wx_r[:],
            wch_d[:, WC_WX:WC_WX + 96].rearrange("(kt p) m -> p kt m", p=P))
        wdt_r = pool.tile([DTR, NT, P], F16, tag="wdt")
        nc.sync.dma_start(wdt_r[:], wdt_d.rearrange("k (mt m) -> k mt m", m=P))
        sm = pool.tile([P, SMALLW], F32, tag="sm")
        nc.sync.dma_start(sm[:], sm_d[:])

        # depthwise-conv diagonal weights, built on device
        dmask = pool.tile([P, P], F16, tag="dmask")
        nc.gpsimd.memset(dmask[:], 1.0)
        nc.gpsimd.affine_select(
            out=dmask[:], in_=dmask[:], compare_op=OP.is_equal, fill=0.0,
            base=0, pattern=[[-1, P]], channel_multiplier=1)
        cdiag = []
        for nt in range(NT):
            cd = pool.tile([P, DC, P], F16, tag=f"cd{nt}", name=f"cd{nt}")
            for k in range(DC):
                nc.vector.tensor_scalar_mul(
                    cd[:, k, :], dmask[:],
                    sm[:, SM_CONVW + nt * DC + k:SM_CONVW + nt * DC + k + 1])
            cdiag.append(cd)

        halo = [pool.tile([P, DC - 1], F16, tag=f"halo{nt}", name=f"halo{nt}")
                for nt in range(NT)]
        states = pool.tile([P, DS * NT], F32, tag="states")

        xt_re = xh_d.rearrange("(kt p) l -> p kt l", p=P)
        st = [dict() for _ in range(NCH)]

        def s1(c):
            """in_proj: xi tiles (mt < NT) first, then z/sz tiles."""
            lc, l0, ltn = CH[c], CHO[c], CH[c] // MMT
            xt_t = []
            for kt in range(KT):
                t = pool.tile([P, lc], F16, tag="big", bufs=8,
                              name=f"xt_{c}_{kt}")
                nc.sync.dma_start(t[:], xt_re[:, kt, l0:l0 + lc])
                xt_t.append(t)
            st[c]["xt_t"] = xt_t
            xi_t, sz_t = [], []
            for mt in range(2 * NT):
                win_t = pool.tile([P, KT, P], F16, tag="win", bufs=4,
                                  name=f"win_{c}_{mt}")
                nc.sync.dma_start(
                    win_t[:],
                    wch_d[:, WC_WIN + mt * P:WC_WIN + (mt + 1) * P].rearrange(
                        "(kt p) q -> p kt q", p=P))
                if mt < NT:
                    xi = pool.tile([P, DC - 1 + lc], F16, tag="xi", bufs=8,
                                   name=f"xi_{c}_{mt}")
                    xi_t.append(xi)
                else:
                    sz = pool.tile([P, lc], F16, tag=f"sz{c}", bufs=NT,
                                   name=f"sz_{c}_{mt}")
                    sz_t.append(sz)
                for lt in range(ltn):
                    acc = psum.tile([P, MMT], F32, tag="mm", bufs=3,
                                    name=f"acc1_{c}_{mt}_{lt}")
                    for kt in range(KT):
                        nc.tensor.matmul(
                            acc[:], win_t[:, kt, :],
                            xt_t[kt][:, lt * MMT:(lt + 1) * MMT],
                            start=(kt == 0), stop=(kt == KT - 1))
                    if mt < NT:
                        nc.scalar.copy(
                            xi_t[mt][:, DC - 1 + lt * MMT:DC - 1 + (lt + 1) * MMT],
                            acc[:])
                    else:
                        nc.scalar.activation(
                            sz_t[mt - NT][:, lt * MMT:(lt + 1) * MMT],
                            acc[:], AF.Silu)
                if mt == NT - 1:
                    st[c].update(xi_t=xi_t)
                yield
            st[c].update(sz_t=sz_t)

        def s23(c):
            """Depthwise conv + bias + silu -> u; x_proj partial -> AllReduce."""
            lc, ltn = CH[c], CH[c] // MMT
            xi_t = st[c]["xi_t"]
            u_t = []
            for nt in range(NT):
                if c == 0:
                    nc.gpsimd.memset(halo[nt][:], 0.0)
                # ACT copies: keep the DVE queue free of background ops
                nc.scalar.copy(xi_t[nt][:, 0:DC - 1], halo[nt][:])
                u = pool.tile([P, lc], F16, tag="xi", bufs=8,
                              name=f"u_{c}_{nt}")
                for lt in range(ltn):
                    acc = psum.tile([P, MMT], F32, tag="mm", bufs=3,
                                    name=f"acc2_{c}_{nt}_{lt}")
                    for k in range(DC):
                        nc.tensor.matmul(
                            acc[:], cdiag[nt][:, k, :],
                            xi_t[nt][:, lt * MMT + k:lt * MMT + k + MMT],
                            start=(k == 0), stop=(k == DC - 1))
                    nc.scalar.activation(
                        u[:, lt * MMT:(lt + 1) * MMT], acc[:], AF.Silu,
                        bias=sm[:, SM_CONVB + nt:SM_CONVB + nt + 1])
                # save halo for the next chunk (before xi slot recycles)
                nc.scalar.copy(
                    halo[nt][:], xi_t[nt][:, lc:lc + DC - 1])
                u_t.append(u)
                yield
            # x_proj partial [96, lc] -> pairwise AllReduce (async)
            xdblp = pool.tile([96, lc], F16, tag="xdblp", bufs=2,
                              name=f"xdblp_{c}")
            for lt in range(ltn):
                acc96 = psum.tile([96, MMT], F32, tag="mm96", bufs=1,
                                  name=f"acc96_{c}_{lt}")
                for nt in range(NT):
                    nc.tensor.matmul(
                        acc96[:], wx_r[:, nt, :],
                        u_t[nt][:, lt * MMT:(lt + 1) * MMT],
                        start=(nt == 0), stop=(nt == NT - 1))
                nc.scalar.copy(xdblp[:, lt * MMT:(lt + 1) * MMT], acc96[:])
                yield
            dma_in = nc.sync.dma_start(ccin[c][:], xdblp[:])
            cc = nc.gpsimd.collective_compute(
                "AllReduce", OP.add, replica_groups=pairs,
                ins=[ccin[c][:]], outs=[ccout[c][:]])
            tile.add_dep_helper(cc.ins, dma_in.ins, reason="cc after dma_in")
            st[c].update(u_t=u_t, cc=cc)
            yield

        def s5(c):
            """dt = softplus(Wdt@dtr + b); dtu = dt*u; y = D*u."""
            lc, ltn = CH[c], CH[c] // MMT
            u_t = st[c]["u_t"]
            xdbl = pool.tile([96, lc], F16, tag="xdbl", bufs=2,
                             name=f"xdbl_{c}")
            dma_out = nc.sync.dma_start(xdbl[:], ccout[c][:])
            tile.add_dep_helper(dma_out.ins, st[c]["cc"].ins,
                                reason="read after cc")
            st[c]["xdbl"] = xdbl
            yield
            dt_t, dtu_t, y_t = [], [], []
            for nt in range(NT):
                dt = pool.tile([P, lc], F16, tag="dt", bufs=8,
                               name=f"dt_{c}_{nt}")
                for lt in range(ltn):
                    acc = psum.tile([P, MMT], F32, tag="mm", bufs=3,
                                    name=f"acc5_{c}_{nt}_{lt}")
                    nc.tensor.matmul(
                        acc[:], wdt_r[:, nt, :],
                        xdbl[0:DTR, lt * MMT:(lt + 1) * MMT],
                        start=True, stop=True)
                    e = pool.tile([P, MMT], F32, tag="spe", bufs=1,
                                  name=f"spe_{c}_{nt}_{lt}")
                    nc.scalar.activation(e[:], acc[:], AF.Exp,
                                         bias=sm[:, SM_DTB + nt:SM_DTB + nt + 1])
                    nc.scalar.activation(
                        dt[:, lt * MMT:(lt + 1) * MMT], e[:], AF.Ln, bias=1.0)
                dt_t.append(dt)
                y = pool.tile([P, lc], F16, tag="y", bufs=8,
                              name=f"y_{c}_{nt}")
                nc.scalar.mul(y[:], u_t[nt][:],
                              sm[:, SM_DCOL + nt:SM_DCOL + nt + 1])  # y = D*u
                y_t.append(y)
                yield
            st[c].update(dt_t=dt_t, y_t=y_t)

        def s6(c):
            """Selective scan; y accumulated on the PE via identity matmuls.
            dA on ACT (fp16), everything elementwise on the DVE."""
            lc, ltn = CH[c], CH[c] // MMT
            dt_t, y_t = st[c]["dt_t"], st[c]["y_t"]
            u_t = st[c]["u_t"]
            # dtu prologue on the DVE (foreground, after s5 fully drained)
            dtu_t = []
            for nt in range(NT):
                dtu = pool.tile([P, lc], F16, tag="dtu", bufs=8,
                                name=f"dtu_{c}_{nt}")
                nc.vector.tensor_tensor(dtu[:], dt_t[nt][:], u_t[nt][:],
                                        OP.mult)
                dtu_t.append(dtu)
            yield
            ynew_t = [None] * NT
            LAG = 2
            PF = 2
            for g in range(NT // 2):
                nts = (2 * g, 2 * g + 1)
                yp = [[psum.tile([P, MMT], F32, tag=f"yp{j}{lt}", bufs=1,
                                 name=f"yp_{c}_{g}_{j}_{lt}")
                       for lt in range(ltn)] for j in range(2)]
                for j, nt in enumerate(nts):
                    for lt in range(ltn):
                        nc.tensor.matmul(
                            yp[j][lt][:], dmask[:],
                            y_t[nt][:, lt * MMT:(lt + 1) * MMT],
                            start=True, stop=False, skip_group_check=True)
                bb_q, cb_q, h_q = {}, {}, {}

                def bcast(n):
                    bb = pool.tile([P, lc], F16, tag="bb", bufs=PF + 2,
                                   name=f"bb_{c}_{g}_{n}")
                    nc.sync.dma_start(
                        bb[:], ccout[c][DTR + n:DTR + n + 1, :]
                        .partition_broadcast(P))
                    cb = pool.tile([P, lc], F16, tag="cb", bufs=PF + LAG + 1,
                                   name=f"cb_{c}_{g}_{n}")
                    nc.sync.dma_start(
                        cb[:], ccout[c][DTR + DS + n:DTR + DS + n + 1, :]
                        .partition_broadcast(P))
                    bb_q[n], cb_q[n] = bb, cb

                for n in range(min(PF, DS)):
                    bcast(n)
                for n in range(DS + LAG):
                    if n < DS:
                        if n + PF < DS:
                            bcast(n + PF)
                        h_t = []
                        for j, nt in enumerate(nts):
                            dA = pool.tile([P, lc], F16, tag="dA", bufs=3,
                                           name=f"dA_{c}_{g}_{n}_{j}")
                            nc.scalar.activation(
                                dA[:], dt_t[nt][:], AF.Exp,
                                scale=sm[:, SM_A + nt * DS + n:
                                         SM_A + nt * DS + n + 1])
                            dBu = pool.tile([P, lc], F16, tag="dbu", bufs=3,
                                            name=f"dbu_{c}_{g}_{n}_{j}")
                            nc.vector.tensor_tensor(dBu[:], dtu_t[nt][:],
                                                    bb_q[n][:], OP.mult)
                            h_t.append((dA, dBu))
                        for j, nt in enumerate(nts):
                            dA, dBu = h_t[j]
                            init = 0.0 if c == 0 else states[:, n * NT + nt:
                                                             n * NT + nt + 1]
                            h = pool.tile([P, lc], F16, tag="h16", bufs=6,
                                          name=f"h_{c}_{g}_{n}_{j}")
                            nc.vector.tensor_tensor_scan(
                                h[:], dA[:], dBu[:], init, OP.mult, OP.add)
                            h_t[j] = h
                            if c < NCH - 1:
                                nc.scalar.copy(
                                    states[:, n * NT + nt:n * NT + nt + 1],
                                    h[:, lc - 1:lc])
                        h_q[n] = h_t
                    m = n - LAG
                    if m >= 0:
                        for j, nt in enumerate(nts):
                            h = h_q[m][j]
                            tmp = pool.tile([P, lc], F16, tag="tmp16", bufs=6,
                                            name=f"tmp_{c}_{g}_{m}_{j}")
                            nc.vector.tensor_tensor(tmp[:], h[:], cb_q[m][:],
                                                    OP.mult)
                            for lt in range(ltn):
                                nc.tensor.matmul(
                                    yp[j][lt][:], dmask[:],
                                    tmp[:, lt * MMT:(lt + 1) * MMT],
                                    start=False, stop=(m == DS - 1),
                                    skip_group_check=True)
                        h_q.pop(m)
                        bb_q.pop(m, None)
                        cb_q.pop(m)
                for j, nt in enumerate(nts):
                    ynew = pool.tile([P, lc], F16, tag="y", bufs=8,
                                     name=f"ynew_{c}_{g}_{j}")
                    for lt in range(ltn):
                        nc.scalar.copy(ynew[:, lt * MMT:(lt + 1) * MMT],
                                       yp[j][lt][:])
                    ynew_t[nts[j]] = ynew
                # yield only at group boundaries: all PSUM accumulation
                # groups are closed here, so interleaved background matmuls
                # never land inside an open group.
                yield
            st[c]["y_t"] = ynew_t

        def s78(c):
            """Gate (in place into sz) + out_proj partial -> output slice."""
            lc, l0, ltn = CH[c], CHO[c], CH[c] // MMT
            y_t, sz_t = st[c]["y_t"], st[c]["sz_t"]
            for nt in range(NT):
                nc.vector.tensor_tensor(sz_t[nt][:], y_t[nt][:], sz_t[nt][:],
                                        OP.mult)
            yield
            for mt in range(KT):
                wout_t = pool.tile([P, NT, P], F16, tag="wout", bufs=4,
                                   name=f"wout_{c}_{mt}")
                nc.sync.dma_start(
                    wout_t[:],
                    wch_d[:, WC_WOUT + mt * P:WC_WOUT + (mt + 1) * P].rearrange(
                        "(kt p) q -> p kt q", p=P))
                for lt in range(ltn):
                    acc = psum.tile([P, MMT], F32, tag="mm", bufs=3,
                                    name=f"acc8_{c}_{mt}_{lt}")
                    for kt in range(NT):
                        nc.tensor.matmul(
                            acc[:], wout_t[:, kt, :],
                            sz_t[kt][:, lt * MMT:(lt + 1) * MMT],
                            start=(kt == 0), stop=(kt == NT - 1))
                    o = pool.tile([P, MMT], F16, tag="op", bufs=2,
                                  name=f"o_{c}_{mt}_{lt}")
                    nc.scalar.copy(o[:], acc[:])
                    nc.sync.dma_start(
                        outp_d[mt * P:(mt + 1) * P,
                               l0 + lt * MMT:l0 + (lt + 1) * MMT], o[:])
                yield

        # ---- schedule: phase-level sequencing (deadlock-free: every op's
        # deps point backward in its engine queue).  Overlap comes from the
        # engine queues draining asynchronously: chunk c+1's in_proj/conv
        # matmuls are queued before chunk c's scan-window yp matmuls, so the
        # PE runs them while the DVE works through chunk c's scans.
        g1 = [s1(c) for c in range(NCH)]
        drive(g1[0], NT)                 # xi in_proj of chunk 0
        drive_all(s23(0))                # conv + x_proj + AllReduce
        drive_all(s5(0))                 # dt/y-seeds of chunk 0
        interleave(s6(0),
                   [g1[0],               # z/sz of chunk 0
                    g1[1],               # in_proj of chunk 1
                    s23(1)], 9)
        drive_all(s5(1))
        interleave(s6(1),
                   [g1[2],               # in_proj of chunk 2
                    s23(2)], 7)
        drive_all(s5(2))
        drive_all(s78(0))
        interleave(s6(2),
                   [s78(1)], 3)
        drive_all(s78(2))

    split_multiwaits(nc)
    return nc


# ------------------------------------------------------------- host side
def _prep_core_inputs(inputs, b, dir_, half):
    pre = "f_" if dir_ == 0 else "b_"
    x = np.asarray(inputs["x"][b], dtype=np.float32)          # [L, DM]
    if dir_ == 1:
        x = x[::-1]
    sl = slice(half * DH, (half + 1) * DH)

    w_in_full = np.asarray(inputs[pre + "in_proj_w"], np.float32)  # [2DI, DM]
    w_in = np.concatenate([w_in_full[sl], w_in_full[DI + half * DH:
                                                    DI + (half + 1) * DH]], 0)
    conv_w = np.asarray(inputs[pre + "conv_w"], np.float32)[sl, 0]  # [DH, DC]
    conv_b = np.asarray(inputs[pre + "conv_b"], np.float32)[sl]
    w_x = np.asarray(inputs[pre + "x_proj_w"], np.float32)[:, sl]   # [96, DH]
    w_dt = np.asarray(inputs[pre + "dt_proj_w"], np.float32)[sl]    # [DH, DTR]
    dt_b = np.asarray(inputs[pre + "dt_proj_b"], np.float32)[sl]
    A = -np.exp(np.asarray(inputs[pre + "A_log"], np.float32))[sl]  # [DH, DS]
    Dp = np.asarray(inputs[pre + "D"], np.float32)[sl]
    w_out = np.asarray(inputs[pre + "out_proj_w"], np.float32)[:, sl]  # [DM,DH]

    smalls = np.zeros((P, SMALLW), np.float32)
    smalls[:, SM_CONVB:SM_CONVB + NT] = conv_b.reshape(NT, P).T
    smalls[:, SM_DTB:SM_DTB + NT] = dt_b.reshape(NT, P).T
    smalls[:, SM_DCOL:SM_DCOL + NT] = Dp.reshape(NT, P).T
    smalls[:, SM_A:SM_A + NT * DS] = (
        A.reshape(NT, P, DS).transpose(1, 0, 2).reshape(P, NT * DS))
    smalls[:, SM_CONVW:SM_CONVW + NT * DC] = (
        conv_w.reshape(NT, P, DC).transpose(1, 0, 2).reshape(P, NT * DC))

    wcat = np.concatenate([w_in.T, w_out.T, w_x.T], axis=1)   # [DM, WCATW]
    xt = x.T                                                  # [DM, L]
    pk = np.concatenate([
        np.ascontiguousarray(xt).astype(np.float16).ravel(),
        np.ascontiguousarray(wcat).astype(np.float16).ravel(),
        np.ascontiguousarray(w_dt.T).astype(np.float16).ravel(),
    ])
    return {"pk": pk, "smalls": smalls}


_CACHE = {}


def _get_nc():
    if "nc" not in _CACHE:
        _CACHE["nc"] = build_nc()
    return _CACHE["nc"]


def _make_runner():
    """Jitted 8-core PJRT runner."""
    import jax
    from jax.sharding import Mesh, PartitionSpec
    from jax.experimental.shard_map import shard_map
    from concourse import bass2jax
    from concourse.bass2jax import _bass_exec_p, install_neuronx_cc_hook

    install_neuronx_cc_hook()
    nc = _get_nc()
    pname = nc.partition_id_tensor.name if nc.partition_id_tensor else None
    in_names, out_names, out_avals = [], [], []
    for alloc in nc.m.functions[0].allocations:
        if not isinstance(alloc, mybir.MemoryLocationSet):
            continue
        name = alloc.memorylocations[0].name
        if alloc.kind == "ExternalInput":
            if name != pname:
                in_names.append(name)
        elif alloc.kind == "ExternalOutput":
            out_names.append(name)
            out_avals.append(jax.core.ShapedArray(
                tuple(alloc.tensor_shape), mybir.dt.np(alloc.dtype)))
    all_names = in_names
    if pname is not None:
        all_names = all_names + [pname]

    def _body(*args):
        operands = list(args)
        if pname is not None:
            operands.append(bass2jax.partition_id_tensor())
        outs = _bass_exec_p.bind(
            *operands, out_avals=tuple(out_avals), in_names=tuple(all_names),
            out_names=tuple(out_names), lowering_input_output_aliases=(),
            sim_require_finite=False, sim_require_nnan=False, nc=nc)
        return tuple(outs)

    devices = jax.devices()[:8]
    mesh = Mesh(np.asarray(devices), ("core",))
    nin = len(in_names)
    fn = jax.jit(shard_map(
        _body, mesh=mesh, in_specs=(PartitionSpec("core"),) * nin,
        out_specs=(PartitionSpec("core"),) * len(out_names), check_rep=False),
        keep_unused=True)
    return fn, in_names, out_names, out_avals


def _get_runner():
    if "runner" not in _CACHE:
        _CACHE["runner"] = _make_runner()
    return _CACHE["runner"]


def _concat_inputs(in_maps):
    import jax
    from jax.sharding import Mesh, NamedSharding, PartitionSpec
    fn, in_names, out_names, out_avals = _get_runner()
    concat = [np.concatenate([np.asarray(m[k]) for m in in_maps], axis=0)
              for k in in_names]
    mesh = Mesh(np.asarray(jax.devices()[:8]), ("core",))
    shard = NamedSharding(mesh, PartitionSpec("core"))
    return [jax.device_put(a, shard) for a in concat]


def _run(in_maps):
    import jax
    fn, in_names, out_names, out_avals = _get_runner()
    args = _concat_inputs(in_maps)
    outs = [np.asarray(o) for o in fn(*args)]
    return [
        {k: outs[i].reshape(8, *out_avals[i].shape)[c]
         for i, k in enumerate(out_names)}
        for c in range(8)
    ]


def run_timed(in_maps, iters=5):
    """Steady-state per-invocation time: issue a batch of executions
    back-to-back, block once, divide. Min over rounds."""
    import time as _t
    import jax
    fn, *_ = _get_runner()
    args = _concat_inputs(in_maps)
    args2 = _concat_inputs(in_maps)
    jax.block_until_ready(fn(*args))
    batch = max(iters, 1536)
    best = float("inf")
    for _ in range(4):
        try:
            t0 = _t.perf_counter()
            o = None
            for i in range(batch):
                o = fn(*(args if i % 2 == 0 else args2))
            jax.block_until_ready(o)
            best = min(best, (_t.perf_counter() - t0) / batch)
        except Exception:
            if best != float("inf"):
                break
            raise
    return best


def make_in_maps(inputs):
    return [
        _prep_core_inputs(inputs, c >> 2, (c >> 1) & 1, c & 1)
        for c in range(8)
    ]


def kernel(**inputs):
    in_maps = make_in_maps(inputs)
    res = _run(in_maps)
    # guard against a rare first-call collective-init flake: run twice and
    # retry while the two executions disagree materially.
    for _ in range(3):
        res2 = _run(in_maps)
        d = max(np.abs(res[c]["outp"].astype(np.float32)
                       - res2[c]["outp"].astype(np.float32)).max()
                for c in range(8))
        if d < 1e-3:
            break
        res = res2
    out = np.zeros((B, L, 2 * DM), np.float32)
    for b in range(B):
        for dir_ in range(2):
            c0 = (b << 2) | (dir_ << 1)
            part = (res[c0]["outp"].astype(np.float32)
                    + res[c0 + 1]["outp"].astype(np.float32))  # [DM, L]
            if dir_ == 1:
                part = part[:, ::-1]
            out[b, :, dir_ * DM:(dir_ + 1) * DM] = part.T
    return out


# revision 35
# speedup vs baseline: 1.1260x; 1.0105x over previous
"""BiMamba (bidirectional Mamba-1 selective scan) on 8 Trainium2 NeuronCores.

Sharding: core c = (b, dir, half) with b = c>>2, dir = (c>>1)&1, half = c&1.
Each core computes its half of d_inner for one (batch, direction) in a
transposed [d, L] layout, fp16 matmul inputs / fp32 accumulation.

L is processed in chunks [512, 1024, 512] with per-(d,n) state carry so the
DVE scan stream starts early and the out_proj tail is short.  Phases are
generators interleaved by an explicit schedule: while the DVE works through
chunk c's scans, the PE runs chunk c+1's in_proj/conv/x_proj and chunk
c-1's out_proj.  All elementwise multiplies (dBu, h*C, dtu, gate) run on
the DVE: the GPSIMD shares SBUF ports with it, and any concurrent Pool
tensor op slows the DVE ~4x (scans ~1.9x), so the Pool is left idle.
  per chunk: in_proj -> depthwise conv (diagonal-weight matmuls) -> silu ->
  x_proj partial -> pairwise AllReduce of x_dbl (f16) -> dt softplus ->
  scan: dA = exp(A*dt) fp16 on ACT, dBu = dtu*B on DVE,
        h = tensor_tensor_scan on DVE, tmp = h*C on DVE,
        y accumulated on the PE with identity matmuls into PSUM
  -> gate with silu(z) in place -> out_proj partial -> output slice.
Host sums the pairwise partial outputs, transposes, and flips bwd.
"""
import sys
sys.path.insert(0, "/opt/trn_rl_repo")
import numpy as np
from contextlib import ExitStack

import concourse.bass as bass
import concourse.mybir as mybir
import concourse.tile as tile
from concourse.vector_clock import ScopedClock

F32 = mybir.dt.float32
F16 = mybir.dt.float16
AF = mybir.ActivationFunctionType
OP = mybir.AluOpType

# ---------------------------------------------------------------- geometry
B, L, DM = 2, 2048, 1024
DI, DS, DC, DTR = 2 * DM, 16, 4, DM // 16
DH = DI // 2              # d_inner half per core
NT = DH // 128            # d-tiles per core
MMT = 512                 # matmul free-dim tile
P = 128
KT = DM // P              # d_model tiles

CH = [512, 1024, 512]     # L chunks (small head -> scans start early)
NCH = len(CH)
CHO = [sum(CH[:i]) for i in range(NCH)]   # chunk offsets
LCMAX = max(CH)

# smalls packing (columns of the [128, SMALLW] f32 tensor)
SM_CONVB = 0              # NT cols
SM_DTB = SM_CONVB + NT    # NT cols
SM_DCOL = SM_DTB + NT     # NT cols
SM_A = SM_DCOL + NT       # NT*DS cols
SM_CONVW = SM_A + NT * DS # NT*DC cols
SMALLW = SM_CONVW + NT * DC

# wcat packing (columns of the [DM, WCATW] f16 tensor): w_in | w_out | w_x
WC_WIN = 0                # 2*DH cols
WC_WOUT = WC_WIN + 2 * DH # DM cols
WC_WX = WC_WOUT + DM      # 96 cols
WCATW = WC_WX + 96

MAXW = 1                  # codegen limit: sem waits per instruction


# ------------------------------------------------------------- tile patch
def _patched_drain_and_barrier(self, tick_clock, wait_clock):
    nop_inst = self.nc.sync.nop(nofuse=True)
    wait_clock.add_sem_waits(
        nop_inst.ins, ScopedClock({None: tick_clock.global_clock}))
    si = nop_inst.ins.sync_info
    if si is not None and si.on_wait and len(si.on_wait) > MAXW:
        extra = list(si.on_wait[MAXW:])
        del si.on_wait[MAXW:]
        for i in range(0, len(extra), MAXW):
            nop2 = self.nc.sync.nop(nofuse=True)
            nop2.ins.sync_info = mybir.SyncInfo(
                on_wait=extra[i:i + MAXW], on_update=[])
    self.nc.sync.drain()
    self.nc.all_engine_barrier()
    assert self.sems is not None
    popped = self.nc._tile_sem_poison_stack.pop()
    assert popped is self._sem_poison
    self.nc.clear_and_free_semaphores(list(self.sems.allocated().values()))
    self.nc.all_engine_barrier()


tile.TileContext._drain_and_barrier = _patched_drain_and_barrier


def split_multiwaits(nc, maxw=MAXW):
    ctr = 0
    for fn in nc.m.functions:
        for blk in fn.blocks:
            il = list(blk.instructions)
            out = []
            changed = False
            for ins in il:
                si = getattr(ins, "sync_info", None)
                waits = list(si.on_wait) if (si is not None and si.on_wait) else []
                if len(waits) > maxw:
                    changed = True
                    extra, keep = waits[:-maxw], waits[-maxw:]
                    for i in range(0, len(extra), maxw):
                        nop = mybir.InstNoOp(name=f"wsplit_{ctr}", ins=[], outs=[])
                        ctr += 1
                        nop.engine = ins.engine
                        nop.sync_info = mybir.SyncInfo(
                            on_wait=extra[i:i + maxw], on_update=[])
                        out.append(nop)
                    si.on_wait = keep
                out.append(ins)
            if changed:
                blk.instructions = out
    return ctr


def drive(gen, n):
    for _ in range(n):
        if next(gen, StopIteration) is StopIteration:
            return False
    return True


def drive_all(gen):
    for _ in gen:
        pass


def interleave(fg, bg_gens, per_yield):
    """Advance up to per_yield background units at each foreground yield,
    draining any leftovers afterwards.

    The foreground (s6) yields only at PSUM-group boundaries, so background
    matmuls are never issued inside an open accumulation group and every
    background op's dependencies resolve strictly backward in each engine
    queue (no cross-engine forward waits -> no deadlock)."""
    import itertools
    bg = itertools.chain(*bg_gens)
    for _ in fg:
        for _ in range(per_yield):
            next(bg, None)
    for _ in bg:
        pass


# ------------------------------------------------------------ bass builder
def build_nc():
    nc = bass.Bass()

    XH_N = DM * L
    WCH_N = DM * WCATW
    WDT_N = DTR * DH
    pk_d = nc.declare_dram_parameter("pk", [XH_N + WCH_N + WDT_N], F16,
                                     isOutput=False)
    xh_d = pk_d[0:XH_N].rearrange("(r c) -> r c", c=L)
    wch_d = pk_d[XH_N:XH_N + WCH_N].rearrange("(r c) -> r c", c=WCATW)
    wdt_d = pk_d[XH_N + WCH_N:XH_N + WCH_N + WDT_N].rearrange(
        "(k c) -> k c", c=DH)
    sm_d = nc.declare_dram_parameter("smalls", [P, SMALLW], F32, isOutput=False)
    outp_d = nc.declare_dram_parameter("outp", [DM, L], F16, isOutput=True)

    ccin = [nc.dram_tensor(f"ccin{c}", [96, CH[c]], F16) for c in range(NCH)]
    ccout = [nc.dram_tensor(f"ccout{c}", [96, CH[c]], F16) for c in range(NCH)]
    pairs = [[0, 1], [2, 3], [4, 5], [6, 7]]

    with tile.TileContext(nc) as tc, ExitStack() as ctx:
        pool = ctx.enter_context(tc.tile_pool(name="sb", bufs=1))
        psum = ctx.enter_context(tc.tile_pool(name="ps", bufs=6, space="PSUM"))

        # resident small weights
        wx_r = pool.tile([P, NT, 96], F16, tag="wx")
        nc.sync.dma_start(
            wx_r[:],
            wch_d[:, WC_WX:WC_WX + 96].rearrange("(kt p) m -> p kt m", p=P))
        wdt_r = pool.tile([DTR, NT, P], F16, tag="wdt")
        nc.sync.dma_start(wdt_r[:], wdt_d.rearrange("k (mt m) -> k mt m", m=P))
        sm = pool.tile([P, SMALLW], F32, tag="sm")
        nc.sync.dma_start(sm[:], sm_d[:])

        # depthwise-conv diagonal weights, built on device
        dmask = pool.tile([P, P], F16, tag="dmask")
        nc.gpsimd.memset(dmask[:], 1.0)
        nc.gpsimd.affine_select(
            out=dmask[:], in_=dmask[:], compare_op=OP.is_equal, fill=0.0,
            base=0, pattern=[[-1, P]], channel_multiplier=1)
        cdiag = []
        for nt in range(NT):
            cd = pool.tile([P, DC, P], F16, tag=f"cd{nt}", name=f"cd{nt}")
            for k in range(DC):
                nc.vector.tensor_scalar_mul(
                    cd[:, k, :], dmask[:],
                    sm[:, SM_CONVW + nt * DC + k:SM_CONVW + nt * DC + k + 1])
            cdiag.append(cd)

        halo = [pool.tile([P, DC - 1], F16, tag=f"halo{nt}", name=f"halo{nt}")
                for nt in range(NT)]
        states = pool.tile([P, DS * NT], F32, tag="states")

        xt_re = xh_d.rearrange("(kt p) l -> p kt l", p=P)
        st = [dict() for _ in range(NCH)]

        def s1(c):
            """in_proj: xi tiles (mt < NT) first, then z/sz tiles."""
            lc, l0, ltn = CH[c], CHO[c], CH[c] // MMT
            xt_t = []
            for kt in range(KT):
                t = pool.tile([P, lc], F16, tag="big", bufs=8,
                              name=f"xt_{c}_{kt}")
                nc.sync.dma_start(t[:], xt_re[:, kt, l0:l0 + lc])
                xt_t.append(t)
            st[c]["xt_t"] = xt_t
            xi_t, sz_t = [], []
            for mt in range(2 * NT):
                win_t = pool.tile([P, KT, P], F16, tag="win", bufs=4,
                                  name=f"win_{c}_{mt}")
                nc.sync.dma_start(
                    win_t[:],
                    wch_d[:, WC_WIN + mt * P:WC_WIN + (mt + 1) * P].rearrange(
                        "(kt p) q -> p kt q", p=P))
                if mt < NT:
                    xi = pool.tile([P, DC - 1 + lc], F16, tag="xi", bufs=8,
                                   name=f"xi_{c}_{mt}")
                    xi_t.append(xi)
                else:
                    sz = pool.tile([P, lc], F16, tag=f"sz{c}", bufs=NT,
                                   name=f"sz_{c}_{mt}")
                    sz_t.append(sz)
                for lt in range(ltn):
                    acc = psum.tile([P, MMT], F32, tag="mm", bufs=3,
                                    name=f"acc1_{c}_{mt}_{lt}")
                    for kt in range(KT):
                        nc.tensor.matmul(
                            acc[:], win_t[:, kt, :],
                            xt_t[kt][:, lt * MMT:(lt + 1) * MMT],
                            start=(kt == 0), stop=(kt == KT - 1))
                    if mt < NT:
                        nc.scalar.copy(
                            xi_t[mt][:, DC - 1 + lt * MMT:DC - 1 + (lt + 1) * MMT],
                            acc[:])
                    else:
                        nc.scalar.activation(
                            sz_t[mt - NT][:, lt * MMT:(lt + 1) * MMT],
                            acc[:], AF.Silu)
                if mt == NT - 1:
                    st[c].update(xi_t=xi_t)
                yield
            st[c].update(sz_t=sz_t)

        def s23(c):
            """Depthwise conv + bias + silu -> u; x_proj partial -> AllReduce."""
            lc, ltn = CH[c], CH[c] // MMT
            xi_t = st[c]["xi_t"]
            u_t = []
            for nt in range(NT):
                if c == 0:
                    nc.gpsimd.memset(halo[nt][:], 0.0)
                # ACT copies: keep the DVE queue free of background ops
                nc.scalar.copy(xi_t[nt][:, 0:DC - 1], halo[nt][:])
                u = pool.tile([P, lc], F16, tag="xi", bufs=8,
                              name=f"u_{c}_{nt}")
                for lt in range(ltn):
                    acc = psum.tile([P, MMT], F32, tag="mm", bufs=3,
                                    name=f"acc2_{c}_{nt}_{lt}")
                    for k in range(DC):
                        nc.tensor.matmul(
                            acc[:], cdiag[nt][:, k, :],
                            xi_t[nt][:, lt * MMT + k:lt * MMT + k + MMT],
                            start=(k == 0), stop=(k == DC - 1))
                    nc.scalar.activation(
                        u[:, lt * MMT:(lt + 1) * MMT], acc[:], AF.Silu,
                        bias=sm[:, SM_CONVB + nt:SM_CONVB + nt + 1])
                # save halo for the next chunk (before xi slot recycles)
                nc.scalar.copy(
                    halo[nt][:], xi_t[nt][:, lc:lc + DC - 1])
                u_t.append(u)
                yield
            # x_proj partial [96, lc] -> pairwise AllReduce (async)
            xdblp = pool.tile([96, lc], F16, tag="xdblp", bufs=2,
                              name=f"xdblp_{c}")
            for lt in range(ltn):
                acc96 = psum.tile([96, MMT], F32, tag="mm96", bufs=1,
                                  name=f"acc96_{c}_{lt}")
                for nt in range(NT):
                    nc.tensor.matmul(
                        acc96[:], wx_r[:, nt, :],
                        u_t[nt][:, lt * MMT:(lt + 1) * MMT],
                        start=(nt == 0), stop=(nt == NT - 1))
                nc.scalar.copy(xdblp[:, lt * MMT:(lt + 1) * MMT], acc96[:])
                yield
            dma_in = nc.sync.dma_start(ccin[c][:], xdblp[:])
            cc = nc.gpsimd.collective_compute(
                "AllReduce", OP.add, replica_groups=pairs,
                ins=[ccin[c][:]], outs=[ccout[c][:]])
            tile.add_dep_helper(cc.ins, dma_in.ins, reason="cc after dma_in")
            st[c].update(u_t=u_t, cc=cc)
            yield

        def s5(c):
            """dt = softplus(Wdt@dtr + b); dtu = dt*u; y = D*u."""
            lc, ltn = CH[c], CH[c] // MMT
            u_t = st[c]["u_t"]
            xdbl = pool.tile([96, lc], F16, tag="xdbl", bufs=2,
                             name=f"xdbl_{c}")
            dma_out = nc.sync.dma_start(xdbl[:], ccout[c][:])
            tile.add_dep_helper(dma_out.ins, st[c]["cc"].ins,
                                reason="read after cc")
            st[c]["xdbl"] = xdbl
            yield
            dt_t, dtu_t, y_t = [], [], []
            for nt in range(NT):
                dt = pool.tile([P, lc], F16, tag="dt", bufs=8,
                               name=f"dt_{c}_{nt}")
                for lt in range(ltn):
                    acc = psum.tile([P, MMT], F32, tag="mm", bufs=3,
                                    name=f"acc5_{c}_{nt}_{lt}")
                    nc.tensor.matmul(
                        acc[:], wdt_r[:, nt, :],
                        xdbl[0:DTR, lt * MMT:(lt + 1) * MMT],
                        start=True, stop=True)
                    e = pool.tile([P, MMT], F32, tag="spe", bufs=1,
                                  name=f"spe_{c}_{nt}_{lt}")
                    nc.scalar.activation(e[:], acc[:], AF.Exp,
                                         bias=sm[:, SM_DTB + nt:SM_DTB + nt + 1])
                    nc.scalar.activation(
                        dt[:, lt * MMT:(lt + 1) * MMT], e[:], AF.Ln, bias=1.0)
                dt_t.append(dt)
                y = pool.tile([P, lc], F16, tag="y", bufs=8,
                              name=f"y_{c}_{nt}")
                nc.scalar.mul(y[:], u_t[nt][:],
                              sm[:, SM_DCOL + nt:SM_DCOL + nt + 1])  # y = D*u
                y_t.append(y)
                yield
            st[c].update(dt_t=dt_t, y_t=y_t)

        def s6(c):
            """Selective scan; y accumulated on the PE via identity matmuls.
            dA on ACT (fp16), everything elementwise on the DVE."""
            lc, ltn = CH[c], CH[c] // MMT
            dt_t, y_t = st[c]["dt_t"], st[c]["y_t"]
            u_t = st[c]["u_t"]
            # dtu prologue on the DVE (foreground, after s5 fully drained)
            dtu_t = []
            for nt in range(NT):
                dtu = pool.tile([P, lc], F16, tag="dtu", bufs=8,
                                name=f"dtu_{c}_{nt}")
                nc.vector.tensor_tensor(dtu[:], dt_t[nt][:], u_t[nt][:],
                                        OP.mult)
                dtu_t.append(dtu)
            yield
            ynew_t = [None] * NT
            LAG = 2
            PF = 2
            for g in range(NT // 2):
                nts = (2 * g, 2 * g + 1)
                yp = [[psum.tile([P, MMT], F32, tag=f"yp{j}{lt}", bufs=1,
                                 name=f"yp_{c}_{g}_{j}_{lt}")
                       for lt in range(ltn)] for j in range(2)]
                for j, nt in enumerate(nts):
                    for lt in range(ltn):
                        nc.tensor.matmul(
                            yp[j][lt][:], dmask[:],
                            y_t[nt][:, lt * MMT:(lt + 1) * MMT],
                            start=True, stop=False, skip_group_check=True)
                bb_q, cb_q, h_q = {}, {}, {}

                def bcast(n):
                    # bb per state; C broadcasts land in per-PAIR tiles so
                    # the h*C multiply below runs as one [P, 2*lc] op.
                    bb = pool.tile([P, lc], F16, tag="bb", bufs=PF + 1,
                                   name=f"bb_{c}_{g}_{n}")
                    nc.sync.dma_start(
                        bb[:], ccout[c][DTR + n:DTR + n + 1, :]
                        .partition_broadcast(P))
                    bb_q[n] = bb
                    p, k = n // 2, n % 2
                    if k == 0:
                        cb_q[p] = pool.tile([P, 2, lc], F16, tag="cb", bufs=3,
                                            name=f"cb_{c}_{g}_{p}")
                    nc.sync.dma_start(
                        cb_q[p][:, k, :],
                        ccout[c][DTR + DS + n:DTR + DS + n + 1, :]
                        .partition_broadcast(P))

                for n in range(min(PF, DS)):
                    bcast(n)
                for n in range(DS + LAG):
                    if n < DS:
                        if n + PF < DS:
                            bcast(n + PF)
                        p, k = n // 2, n % 2
                        if k == 0:
                            h_q[p] = [
                                pool.tile([P, 2, lc], F16, tag="h16", bufs=3,
                                          name=f"hp_{c}_{g}_{p}_{j}")
                                for j in range(2)]
                        h_t = []
                        for j, nt in enumerate(nts):
                            dA = pool.tile([P, lc], F16, tag="dA", bufs=3,
                                           name=f"dA_{c}_{g}_{n}_{j}")
                            nc.scalar.activation(
                                dA[:], dt_t[nt][:], AF.Exp,
                                scale=sm[:, SM_A + nt * DS + n:
                                         SM_A + nt * DS + n + 1])
                            dBu = pool.tile([P, lc], F16, tag="dbu", bufs=3,
                                            name=f"dbu_{c}_{g}_{n}_{j}")
                            nc.vector.tensor_tensor(dBu[:], dtu_t[nt][:],
                                                    bb_q[n][:], OP.mult)
                            h_t.append((dA, dBu))
                        for j, nt in enumerate(nts):
                            dA, dBu = h_t[j]
                            init = 0.0 if c == 0 else states[:, n * NT + nt:
                                                             n * NT + nt + 1]
                            nc.vector.tensor_tensor_scan(
                                h_q[p][j][:, k, :], dA[:], dBu[:], init,
                                OP.mult, OP.add)
                            if c < NCH - 1:
                                nc.scalar.copy(
                                    states[:, n * NT + nt:n * NT + nt + 1],
                                    h_q[p][j][:, k, lc - 1:lc])
                        bb_q.pop(n - PF, None)
                    m = n - LAG
                    if m >= 0 and m % 2 == 1:
                        p = m // 2
                        for j, nt in enumerate(nts):
                            tmp = pool.tile([P, 2, lc], F16, tag="tmp16",
                                            bufs=3,
                                            name=f"tmp_{c}_{g}_{p}_{j}")
                            nc.vector.tensor_tensor(tmp[:], h_q[p][j][:],
                                                    cb_q[p][:], OP.mult)
                            for k in range(2):
                                for lt in range(ltn):
                                    nc.tensor.matmul(
                                        yp[j][lt][:], dmask[:],
                                        tmp[:, k, lt * MMT:(lt + 1) * MMT],
                                        start=False, stop=(m == DS - 1
                                                           and k == 1),
                                        skip_group_check=True)
                        h_q.pop(p)
                        cb_q.pop(p)
                for j, nt in enumerate(nts):
                    ynew = pool.tile([P, lc], F16, tag="y", bufs=8,
                                     name=f"ynew_{c}_{g}_{j}")
                    for lt in range(ltn):
                        nc.scalar.copy(ynew[:, lt * MMT:(lt + 1) * MMT],
                                       yp[j][lt][:])
                    ynew_t[nts[j]] = ynew
                # yield only at group boundaries: all PSUM accumulation
                # groups are closed here, so interleaved background matmuls
                # never land inside an open group.
                yield
            st[c]["y_t"] = ynew_t

        def s78(c):
            """Gate (in place into sz) + out_proj partial -> output slice."""
            lc, l0, ltn = CH[c], CHO[c], CH[c] // MMT
            y_t, sz_t = st[c]["y_t"], st[c]["sz_t"]
            for nt in range(NT):
                nc.vector.tensor_tensor(sz_t[nt][:], y_t[nt][:], sz_t[nt][:],
                                        OP.mult)
            yield
            for mt in range(KT):
                wout_t = pool.tile([P, NT, P], F16, tag="wout", bufs=4,
                                   name=f"wout_{c}_{mt}")
                nc.sync.dma_start(
                    wout_t[:],
                    wch_d[:, WC_WOUT + mt * P:WC_WOUT + (mt + 1) * P].rearrange(
                        "(kt p) q -> p kt q", p=P))
                for lt in range(ltn):
                    acc = psum.tile([P, MMT], F32, tag="mm", bufs=3,
                                    name=f"acc8_{c}_{mt}_{lt}")
                    for kt in range(NT):
                        nc.tensor.matmul(
                            acc[:], wout_t[:, kt, :],
                            sz_t[kt][:, lt * MMT:(lt + 1) * MMT],
                            start=(kt == 0), stop=(kt == NT - 1))
                    o = pool.tile([P, MMT], F16, tag="op", bufs=2,
                                  name=f"o_{c}_{mt}_{lt}")
                    nc.scalar.copy(o[:], acc[:])
                    nc.sync.dma_start(
                        outp_d[mt * P:(mt + 1) * P,
                               l0 + lt * MMT:l0 + (lt + 1) * MMT], o[:])
                yield

        # ---- schedule: phase-level sequencing (deadlock-free: every op's
        # deps point backward in its engine queue).  Overlap comes from the
        # engine queues draining asynchronously: chunk c+1's in_proj/conv
        # matmuls are queued before chunk c's scan-window yp matmuls, so the
        # PE runs them while the DVE works through chunk c's scans.
        g1 = [s1(c) for c in range(NCH)]
        drive(g1[0], NT)                 # xi in_proj of chunk 0
        drive_all(s23(0))                # conv + x_proj + AllReduce
        drive_all(s5(0))                 # dt/y-seeds of chunk 0
        interleave(s6(0),
                   [g1[0],               # z/sz of chunk 0
                    g1[1],               # in_proj of chunk 1
                    s23(1)], 9)
        drive_all(s5(1))
        interleave(s6(1),
                   [g1[2],               # in_proj of chunk 2
                    s23(2)], 7)
        drive_all(s5(2))
        drive_all(s78(0))
        interleave(s6(2),
                   [s78(1)], 3)
        drive_all(s78(2))

    split_multiwaits(nc)
    return nc


# ------------------------------------------------------------- host side
def _prep_core_inputs(inputs, b, dir_, half):
    pre = "f_" if dir_ == 0 else "b_"
    x = np.asarray(inputs["x"][b], dtype=np.float32)          # [L, DM]
    if dir_ == 1:
        x = x[::-1]
    sl = slice(half * DH, (half + 1) * DH)

    w_in_full = np.asarray(inputs[pre + "in_proj_w"], np.float32)  # [2DI, DM]
    w_in = np.concatenate([w_in_full[sl], w_in_full[DI + half * DH:
                                                    DI + (half + 1) * DH]], 0)
    conv_w = np.asarray(inputs[pre + "conv_w"], np.float32)[sl, 0]  # [DH, DC]
    conv_b = np.asarray(inputs[pre + "conv_b"], np.float32)[sl]
    w_x = np.asarray(inputs[pre + "x_proj_w"], np.float32)[:, sl]   # [96, DH]
    w_dt = np.asarray(inputs[pre + "dt_proj_w"], np.float32)[sl]    # [DH, DTR]
    dt_b = np.asarray(inputs[pre + "dt_proj_b"], np.float32)[sl]
    A = -np.exp(np.asarray(inputs[pre + "A_log"], np.float32))[sl]  # [DH, DS]
    Dp = np.asarray(inputs[pre + "D"], np.float32)[sl]
    w_out = np.asarray(inputs[pre + "out_proj_w"], np.float32)[:, sl]  # [DM,DH]

    smalls = np.zeros((P, SMALLW), np.float32)
    smalls[:, SM_CONVB:SM_CONVB + NT] = conv_b.reshape(NT, P).T
    smalls[:, SM_DTB:SM_DTB + NT] = dt_b.reshape(NT, P).T
    smalls[:, SM_DCOL:SM_DCOL + NT] = Dp.reshape(NT, P).T
    smalls[:, SM_A:SM_A + NT * DS] = (
        A.reshape(NT, P, DS).transpose(1, 0, 2).reshape(P, NT * DS))
    smalls[:, SM_CONVW:SM_CONVW + NT * DC] = (
        conv_w.reshape(NT, P, DC).transpose(1, 0, 2).reshape(P, NT * DC))

    wcat = np.concatenate([w_in.T, w_out.T, w_x.T], axis=1)   # [DM, WCATW]
    xt = x.T                                                  # [DM, L]
    pk = np.concatenate([
        np.ascontiguousarray(xt).astype(np.float16).ravel(),
        np.ascontiguousarray(wcat).astype(np.float16).ravel(),
        np.ascontiguousarray(w_dt.T).astype(np.float16).ravel(),
    ])
    return {"pk": pk, "smalls": smalls}


_CACHE = {}


def _get_nc():
    if "nc" not in _CACHE:
        _CACHE["nc"] = build_nc()
    return _CACHE["nc"]


def _make_runner():
    """Jitted 8-core PJRT runner."""
    import jax
    from jax.sharding import Mesh, PartitionSpec
    from jax.experimental.shard_map import shard_map
    from concourse import bass2jax
    from concourse.bass2jax import _bass_exec_p, install_neuronx_cc_hook

    install_neuronx_cc_hook()
    nc = _get_nc()
    pname = nc.partition_id_tensor.name if nc.partition_id_tensor else None
    in_names, out_names, out_avals = [], [], []
    for alloc in nc.m.functions[0].allocations:
        if not isinstance(alloc, mybir.MemoryLocationSet):
            continue
        name = alloc.memorylocations[0].name
        if alloc.kind == "ExternalInput":
            if name != pname:
                in_names.append(name)
        elif alloc.kind == "ExternalOutput":
            out_names.append(name)
            out_avals.append(jax.core.ShapedArray(
                tuple(alloc.tensor_shape), mybir.dt.np(alloc.dtype)))
    all_names = in_names
    if pname is not None:
        all_names = all_names + [pname]

    def _body(*args):
        operands = list(args)
        if pname is not None:
            operands.append(bass2jax.partition_id_tensor())
        outs = _bass_exec_p.bind(
            *operands, out_avals=tuple(out_avals), in_names=tuple(all_names),
            out_names=tuple(out_names), lowering_input_output_aliases=(),
            sim_require_finite=False, sim_require_nnan=False, nc=nc)
        return tuple(outs)

    devices = jax.devices()[:8]
    mesh = Mesh(np.asarray(devices), ("core",))
    nin = len(in_names)
    fn = jax.jit(shard_map(
        _body, mesh=mesh, in_specs=(PartitionSpec("core"),) * nin,
        out_specs=(PartitionSpec("core"),) * len(out_names), check_rep=False),
        keep_unused=True)
    return fn, in_names, out_names, out_avals


def _get_runner():
    if "runner" not in _CACHE:
        _CACHE["runner"] = _make_runner()
    return _CACHE["runner"]


def _concat_inputs(in_maps):
    import jax
    from jax.sharding import Mesh, NamedSharding, PartitionSpec
    fn, in_names, out_names, out_avals = _get_runner()
    concat = [np.concatenate([np.asarray(m[k]) for m in in_maps], axis=0)
              for k in in_names]
    mesh = Mesh(np.asarray(jax.devices()[:8]), ("core",))
    shard = NamedSharding(mesh, PartitionSpec("core"))
    return [jax.device_put(a, shard) for a in concat]


def _run(in_maps):
    import jax
    fn, in_names, out_names, out_avals = _get_runner()
    args = _concat_inputs(in_maps)
    outs = [np.asarray(o) for o in fn(*args)]
    return [
        {k: outs[i].reshape(8, *out_avals[i].shape)[c]
         for i, k in enumerate(out_names)}
        for c in range(8)
    ]


def run_timed(in_maps, iters=5):
    """Steady-state per-invocation time: issue a batch of executions
    back-to-back, block once, divide. Min over rounds."""
    import time as _t
    import jax
    fn, *_ = _get_runner()
    args = _concat_inputs(in_maps)
    args2 = _concat_inputs(in_maps)
    jax.block_until_ready(fn(*args))
    batch = max(iters, 1536)
    best = float("inf")
    for _ in range(4):
        try:
            t0 = _t.perf_counter()
            o = None
            for i in range(batch):
                o = fn(*(args if i % 2 == 0 else args2))
            jax.block_until_ready(o)
            best = min(best, (_t.perf_counter() - t0) / batch)
        except Exception:
            if best != float("inf"):
                break
            raise
    return best


def make_in_maps(inputs):
    return [
        _prep_core_inputs(inputs, c >> 2, (c >> 1) & 1, c & 1)
        for c in range(8)
    ]


def kernel(**inputs):
    in_maps = make_in_maps(inputs)
    res = _run(in_maps)
    # guard against a rare first-call collective-init flake: run twice and
    # retry while the two executions disagree materially.
    for _ in range(3):
        res2 = _run(in_maps)
        d = max(np.abs(res[c]["outp"].astype(np.float32)
                       - res2[c]["outp"].astype(np.float32)).max()
                for c in range(8))
        if d < 1e-3:
            break
        res = res2
    out = np.zeros((B, L, 2 * DM), np.float32)
    for b in range(B):
        for dir_ in range(2):
            c0 = (b << 2) | (dir_ << 1)
            part = (res[c0]["outp"].astype(np.float32)
                    + res[c0 + 1]["outp"].astype(np.float32))  # [DM, L]
            if dir_ == 1:
                part = part[:, ::-1]
            out[b, :, dir_ * DM:(dir_ + 1) * DM] = part.T
    return out
